# revision 15
# baseline (speedup 1.0000x reference)
"""DistinctionLoss Trainium2 kernel (raw bacc, hand-scheduled) — v2.

Math (per batch b):
  f_n = x_n / ||x_n||                       (row-normalized features)
  s   = sum_n f_n                           ([D] weighted row sum)
  mean(gram) = ||s||^2 / N^2                (the N x N gram is never built)
  dot_n = f_n . s = rn_n * (x_n . s)
  sim_n = (dot_n - 1)/(N-1);  t_n = 1 - relu(sim_n)
  bce  = -mean(t*log(sc) + (1-t)*log1p(-sc))   (logs clamped at -100)
  loss = bce + 1 - mean_b(||s_b||^2)/N^2

Sharding: data-parallel over B=8 across 8 NeuronCores (1 batch per core).
Features are cast to bf16 on the host. Each core returns out[1, 2]:
col 0 = sum_n(ls_n - relu(sim_n)*w_n), col 1 = ||s||^2.

Schedule (8 chunks of 4 groups; measured cost models):
  sync: chunks 0-3 DMA, out DMA
  gp  : scores DMA, chunks 4-7 DMA
  ACT : one sqrt_and_others table: squares of chunks {2,3,6,7},
        per-slab Sqrt(1/ssq); then natural_log table: Ln(sc), Ln(1-sc);
        s copy, ||s||^2, final out copy
  DVE : squares of chunks {0,1,4,5}, per-slab fold+reduce -> ssq,
        reciprocal; s broadcast copy; phase-2: 32 fused
        scalar_tensor_tensor (x_g * s_bcast, accum -> draw[:, g]); tail
  PE  : HAM warmups, 32 accumulating matmuls (s), s broadcast, final sum
"""

import numpy as np
import ml_dtypes

B = 8
N, D, P = 4096, 256, 128
G = N // P            # 32 groups per partition
CG = 4                # groups per chunk
NCH = G // CG         # 8 chunks
NINV = 1.0 / (N - 1)
LOG_CLAMP = -100.0

# chunk ownership for the square pass (DVE is 2x rate, ACT 1x)
DVE_SQ = (0, 1, 4, 5)
ACT_SQ = (2, 3, 6, 7)
# slabs of 2 chunks (8 groups) for the fold+reduce
SLABS = ((0, 1), (4, 5), (2, 3), (6, 7))

_cache = {}


def _build_nc():
    import concourse.bacc as bacc
    import concourse.bass as bass
    from concourse import mybir
    from contextlib import ExitStack

    fp32 = mybir.dt.float32
    bf16 = mybir.dt.bfloat16
    AF = mybir.ActivationFunctionType
    ALU = mybir.AluOpType
    AX = mybir.AxisListType

    nc = bacc.Bacc(
        "TRN2", target_bir_lowering=False, debug=False,
        enable_asserts=False, num_devices=8,
    )

    xbf = nc.dram_tensor("xbf", [N, D], bf16, kind="ExternalInput")
    scores = nc.dram_tensor("scores", [N, 1], fp32, kind="ExternalInput")
    out_d = nc.dram_tensor("out", [1, 2], fp32, kind="ExternalOutput")

    x_r = xbf[:].rearrange("(p g) d -> p g d", p=P)
    sc_r = scores[:].rearrange("(p g) o -> p (g o)", p=P)

    sb = nc.alloc_sbuf_tensor
    x_t = sb("x", [P, G, D], bf16)
    sq_t = sb("sq", [P, G, D], bf16)
    f1_t = sb("f1", [P, CG * 2, 128], bf16)      # slab scratch
    f2_t = sb("f2", [P, CG * 2, 64], bf16)
    ssq_t = sb("ssq", [P, G], fp32)
    issq_t = sb("issq", [P, G], fp32)
    rnbf_t = sb("rnbf", [P, G], bf16)
    sc_t = sb("sc", [P, G], fp32)
    ls_t = sb("ls", [P, G], fp32)
    l1_t = sb("l1", [P, G], fp32)
    w_t = sb("w", [P, G], fp32)
    lssum_t = sb("lssum", [P, 1], fp32)
    pt_t = sb("pt", [P, G, D], bf16)             # phase-2 throwaway product
    draw_t = sb("draw", [P, G], fp32)
    dots_t = sb("dots", [P, G], fp32)
    sim_t = sb("sim", [P, G], fp32)
    rterm_t = sb("rterm", [P, G], fp32)
    rwsum_t = sb("rwsum", [P, 1], fp32)
    onesb_t = sb("onesb", [1, P], bf16)
    onesf_t = sb("onesf", [P, 1], fp32)
    sbf1_t = sb("sbf1", [1, D], bf16)
    sbc_t = sb("sbc", [P, D], bf16)
    sscr_t = sb("sscr", [1, D], fp32)
    warm_t = sb("warm", [1, 2], fp32)
    outfin_t = sb("outfin", [1, 2], fp32)
    outsb_t = sb("outsb", [P, 2], fp32)

    ctx = ExitStack()
    ps_s = ctx.enter_context(nc.psum_tensor([1, D], fp32))
    ps_bc = ctx.enter_context(nc.psum_tensor([P, D], fp32))
    ps_tot = ctx.enter_context(nc.psum_tensor([1, 2], fp32))
    names = ([f"S_dx{c}" for c in range(NCH)] +
             ["S_dsc", "S_sqA", "S_red", "S_rn", "S_pe",
              "S_sbf", "S_pebc", "S_ln", "S_dveE", "S_out", "S_pef",
              "S_fin", "S_ones", "S_od"])
    S = {n: ctx.enter_context(nc.semaphore(n)) for n in names}

    # warmup moving operand: onesb viewed as [1, 2, 128] (stride-0 mid dim)
    _ob = onesb_t[:]
    warm_mov_ap = bass.AP(tensor=_ob.tensor, offset=_ob.offset,
                          ap=[_ob.ap[0], [0, 2], _ob.ap[1]])

    def gsl(c):
        return slice(c * CG, (c + 1) * CG)

    # chunk arrival checks: sync does 0..3, gp does 4..7
    def wait_chunk(eng, c):
        eng.wait_ge(S[f"S_dx{c}"], 16)

    # which sq chunks are produced by ACT, and the sem value when chunk c done
    ACT_ORD = {c: i + 1 for i, c in enumerate(ACT_SQ)}

    with ctx, nc.Block() as block:
        @block.sync
        def _(sync):
            for c in range(4):
                sync.dma_start(out=x_t[:, gsl(c), :], in_=x_r[:, gsl(c), :]
                               ).then_inc(S[f"S_dx{c}"], 16)
            sync.wait_ge(S["S_fin"], 1)
            sync.dma_start(out=out_d[:], in_=outfin_t[:]).then_inc(S["S_od"], 16)
            sync.wait_ge(S["S_od"], 16)

        @block.gpsimd
        def _(gp):
            gp.dma_start(out=sc_t[:], in_=sc_r).then_inc(S["S_dsc"], 16)
            for c in range(4, 8):
                gp.dma_start(out=x_t[:, gsl(c), :], in_=x_r[:, gsl(c), :]
                             ).then_inc(S[f"S_dx{c}"], 16)

        @block.scalar
        def _(act):
            # load the sqrt_and_others table (square + sqrt + copy)
            act.activation(out=warm_t[:, 0:1],
                           in_=nc.const_aps.tensor(1.0, (1, 1)), func=AF.Square)
            act.sqrt(warm_t[:, 1:2], nc.const_aps.tensor(1.0, (1, 1)))
            # squares of ACT-owned chunks
            for c in ACT_SQ:
                wait_chunk(act, c)
                act.activation(out=sq_t[:, gsl(c), :], in_=x_t[:, gsl(c), :],
                               func=AF.Square).then_inc(S["S_sqA"], 1)
            # per-slab rn = sqrt(1/ssq) as DVE reduces land
            for si, sl in enumerate(SLABS):
                act.wait_ge(S["S_red"], si + 1)
                g0, g1 = sl[0] * CG, sl[1] * CG + CG
                act.sqrt(rnbf_t[:, g0:g1], issq_t[:, g0:g1]
                         ).then_inc(S["S_rn"], 1)
            # s: PSUM -> SBUF bf16; ||s||^2 -> outsb[0,1]
            act.wait_ge(S["S_pe"], 1)
            act.copy(sbf1_t[:], ps_s[:]).then_inc(S["S_sbf"], 1)
            act.activation(
                out=sscr_t[:], in_=ps_s[:], func=AF.Square,
                accum_out=outsb_t[0:1, 1:2],
            ).then_inc(S["S_out"], 1)
            # natural_log table load happens here (hidden under phase-2)
            act.wait_ge(S["S_dsc"], 16)
            act.activation(out=ls_t[:], in_=sc_t[:], func=AF.Ln)
            act.activation(
                out=l1_t[:], in_=sc_t[:], func=AF.Ln, scale=-1.0, bias=1.0,
            ).then_inc(S["S_ln"], 1)
            act.wait_ge(S["S_pef"], 1)
            act.copy(outfin_t[:], ps_tot[:]).then_inc(S["S_fin"], 1)

        @block.vector
        def _(dve):
            dve.memset(onesb_t[:], 1.0)
            dve.memset(onesf_t[:], 1.0).then_inc(S["S_ones"], 1)
            dve.memset(outsb_t[:], 0.0)
            # squares of DVE-owned chunks (TT mult, 2x bf16)
            for c in (0, 1):
                wait_chunk(dve, c)
                dve.tensor_mul(sq_t[:, gsl(c), :], x_t[:, gsl(c), :],
                               x_t[:, gsl(c), :])
            # slabs in arrival-friendly order; each: f1, f2, reduce, recip
            for si, sl in enumerate(SLABS):
                for c in sl:
                    if c in DVE_SQ and c >= 4:
                        wait_chunk(dve, c)
                        dve.tensor_mul(sq_t[:, gsl(c), :], x_t[:, gsl(c), :],
                                       x_t[:, gsl(c), :])
                    elif c in ACT_ORD:
                        dve.wait_ge(S["S_sqA"], ACT_ORD[c])
                if sl[0] in DVE_SQ:
                    dve.drain()
                g0, g1 = sl[0] * CG, sl[1] * CG + CG
                src = sq_t[:, g0:g1, :]
                dve.tensor_add(f1_t[:], src[:, :, 0:128], src[:, :, 128:256])
                dve.drain()
                dve.tensor_add(f2_t[:], f1_t[:, :, 0:64], f1_t[:, :, 64:128])
                dve.drain()
                dve.tensor_reduce(out=ssq_t[:, g0:g1], in_=f2_t[:],
                                  axis=AX.X, op=ALU.add)
                dve.drain()
                dve.reciprocal(issq_t[:, g0:g1], ssq_t[:, g0:g1]
                               ).then_inc(S["S_red"], 1)
            # s broadcast: PSUM -> SBUF bf16
            dve.wait_ge(S["S_pebc"], 1)
            dve.tensor_copy(sbc_t[:], ps_bc[:])
            dve.drain()
            # phase-2: fused per-group dot with s
            for g in range(G):
                dve.scalar_tensor_tensor(
                    out=pt_t[:, g, :], in0=x_t[:, g, :], scalar=1.0,
                    in1=sbc_t[:], op0=ALU.mult, op1=ALU.mult,
                    accum_out=draw_t[:, g:g + 1],
                )
            # scores tail (Ln lands during the stt loop)
            dve.wait_ge(S["S_ln"], 1)
            dve.tensor_scalar_max(ls_t[:], ls_t[:], LOG_CLAMP)
            dve.drain()
            dve.tensor_sub(w_t[:], ls_t[:], l1_t[:])
            dve.tensor_reduce(out=lssum_t[:], in_=ls_t[:], axis=AX.X, op=ALU.add)
            dve.drain()
            dve.tensor_mul(dots_t[:], draw_t[:], rnbf_t[:])
            dve.drain()
            dve.tensor_scalar(
                out=sim_t[:], in0=dots_t[:], scalar1=1.0, scalar2=NINV,
                op0=ALU.subtract, op1=ALU.mult,
            )
            dve.drain()
            dve.scalar_tensor_tensor(
                out=rterm_t[:], in0=sim_t[:], scalar=0.0, in1=w_t[:],
                op0=ALU.max, op1=ALU.mult, accum_out=rwsum_t[:],
            )
            dve.drain()
            dve.tensor_sub(outsb_t[:, 0:1], lssum_t[:], rwsum_t[:]
                           ).then_inc(S["S_dveE"], 1)

        @block.tensor
        def _(pe):
            # HAM warmup: dummy matmuls so the real ones run at 8/8 rate
            pe.wait_ge(S["S_ones"], 1)
            for _ in range(14):
                pe.matmul(ps_bc[:, 0:D], onesb_t[:], warm_mov_ap,
                          start=True, stop=True)
            mm = None
            done = 0
            for si, sl in enumerate(SLABS):
                pe.wait_ge(S["S_rn"], si + 1)
                for c in sl:
                    for gl in range(CG):
                        g = c * CG + gl
                        mm = pe.matmul(
                            ps_s[:], rnbf_t[:, g:g + 1], x_t[:, g, :],
                            start=(done == 0), stop=(done == G - 1),
                        )
                        done += 1
            mm.then_inc(S["S_pe"], 1)
            pe.wait_ge(S["S_sbf"], 1)
            pe.matmul(ps_bc[:], onesb_t[:], sbf1_t[:], start=True, stop=True
                      ).then_inc(S["S_pebc"], 1)
            pe.wait_ge(S["S_dveE"], 1)
            pe.wait_ge(S["S_out"], 1)
            pe.matmul(ps_tot[:], onesf_t[:], outsb_t[:], start=True, stop=True
                      ).then_inc(S["S_pef"], 1)

    nc.finalize()
    return nc


def _get_nc():
    if "nc" not in _cache:
        _cache["nc"] = _build_nc()
    return _cache["nc"]


def run_on_device(features: np.ndarray, scores: np.ndarray, trace: bool = False,
                  tmpdir: str | None = None):
    """Returns (per_core_outputs [8, 2] float64, BassKernelResults)."""
    from concourse.bass_utils import run_bass_kernel_spmd

    nc = _get_nc()
    in_maps = []
    for c in range(B):
        in_maps.append({
            "xbf": np.ascontiguousarray(features[c]).astype(ml_dtypes.bfloat16),
            "scores": np.ascontiguousarray(scores[c]).astype(np.float32),
        })
    res = run_bass_kernel_spmd(nc, in_maps, core_ids=list(range(B)),
                               trace=trace, tmpdir=tmpdir)
    outs = np.stack([res.results[c]["out"].reshape(2) for c in range(B)])
    return outs.astype(np.float64), res


def kernel(features: np.ndarray, scores: np.ndarray) -> np.ndarray:
    outs, _ = run_on_device(features, scores)
    bce_sums = outs[:, 0]                         # per-batch BCE sums
    ssqs = outs[:, 1]                             # per-batch ||s||^2
    bce = np.mean(-bce_sums / N)
    feat = 1.0 - np.sum(ssqs) / (B * float(N) * float(N))
    return np.asarray(bce + feat, dtype=np.float32)
